# revision 37
# baseline (speedup 1.0000x reference)
"""Trainium2 Bass kernel for nn_MultiHeadAttention_69930657513858.

Single-token (decode) multi-head attention, B=8, E=4096, H=32 heads of
D=128, with a KV cache that is identically ones (length L=4095).

Because the cache is all-ones, attention collapses to a closed form:
  scores = [s0]*L ++ [s1],  s0 = sum_d(q)/sqrt(D), s1 = (q.k)/sqrt(D)
  softmax => p_last = sigmoid(s1 - s0 - ln(L)); cache mass = 1 - p_last
  o = 1 + p_last*(v - 1)
and since s1 - s0 = sum_d q*(k-1), the whole attention needs one
partition-dim reduction.  Furthermore out = o @ Wo^T splits into
rowsum(Wo) (computed exactly on the host, batch-independent) plus
Wo @ delta with delta = p*(v-1), so all device GEMM traffic tolerates
low precision: every weight ships as fp8 e4m3 (scaled 2^10), which is
4x less HBM/DMA traffic than fp32 -- the sole bottleneck of this
memory-bound decode step.  x ships as single fp8; delta is split into
an fp8 hi+lo pair (residual splitting) except on the last 1024 output
columns, whose drain chains gate the final writes; measured output rel
err ~6e-3 vs the 2e-2 gate.

Matmuls keep the WEIGHT stationary and stream the tiny activations
(moving free dim = 8/16), so PE time is ~2us against ~24us of DMA.
Layouts are chosen so the head dim d lands on partitions: q^T/k^T/v^T
tiles are [128d, 4h, 8b], making s=sum_d a ones-vector matmul and the
per-(h,b) sigmoid a [1,32] op; p broadcasts back over partitions with a
rank-1 matmul against a constant-64 row (folding the delta fp8 scale).

Sharding: tensor-parallel over heads, 4 heads per core (Wq/Wk/Wv row
slices, Wo column slices); per-core out-proj partials are summed on the
host together with rowsum(Wo) (the "all-reduce").
"""

import math
import os

import numpy as np

B = 8
E = 4096
H = 32
D = 128
L = 4095
N_CORES = 8
HPC = H // N_CORES  # heads per core = 4
F = HPC * D  # per-core head width = 512
ET = E // 128  # e tiles = 32
SCALE = 1.0 / math.sqrt(D)
BIAS = -math.log(L)

WS = 1024.0  # weight fp8 scale (2^10)
XS = 16.0  # x fp8 scale (2^4)
DS = 64.0  # delta fp8 scale (2^6), folded into the p broadcast
QSC = WS * XS  # q/k/v psum scale (2^14)
OSC = WS * DS  # out psum scale (2^16)

MODE = os.environ.get("MHA_MODE", "fp8")

NWOC = 9  # wo e-chunks (7x512 + 384 + 128 cols)

_CACHE = {}


def _build_program():
    import concourse.mybir as mybir
    import concourse.tile as tile
    from concourse import bacc

    fp32 = mybir.dt.float32
    f8 = mybir.dt.float8e4

    nc = bacc.Bacc("TRN2", target_bir_lowering=False)

    # all HBM operands are packed partition-major on the host so every
    # DMA descriptor is a contiguous >=512B run (full 360GB/s in one shot);
    # x rides in front of wq in one tensor (one DMA, no pipeline gap)
    XW = ET * B  # 256 x bytes per partition (single fp8, no residual)
    wqx = nc.dram_tensor("wqx_t", [128, XW + ET * F], f8, kind="ExternalInput").ap()
    wk = nc.dram_tensor("wk_t", [128, ET * F], f8, kind="ExternalInput").ap()
    wv = nc.dram_tensor("wv_t", [128, ET * F], f8, kind="ExternalInput").ap()
    # wo cols 0..3583 in 512-col chunks; the 384-col and 128-col tail
    # chunks get their own partition-major tensors so their contiguous
    # runs stay >=512B (no 2x DMA latency multiplier).  The 128-col
    # chunk arrives last: its post-DMA chain (4 matmuls + one tiny
    # PSUM drain) is the shortest possible before the final write.
    wo = nc.dram_tensor("wo_t", [128, HPC * 3584], f8, kind="ExternalInput").ap()
    wo7 = nc.dram_tensor("wo7_t", [128, HPC * 384], f8, kind="ExternalInput").ap()
    wo8 = nc.dram_tensor("wo8_t", [128, HPC * 128], f8, kind="ExternalInput").ap()
    out = nc.dram_tensor("out_p", [128, ET * B], fp32, kind="ExternalOutput").ap()

    wk_r = wk.rearrange("p (t f) -> p t f", t=ET)  # [128, 32, 512]
    wv_r = wv.rearrange("p (t f) -> p t f", t=ET)
    wo_r = wo.rearrange("p (t e) -> p t e", t=HPC)  # [128, 4, 3584]
    out_r = out.rearrange("p (t b) -> p t b", t=ET)  # [128, 32, 8]

    with tile.TileContext(nc) as tc:
        with (
            tc.tile_pool(name="const", bufs=1) as const_pool,
            tc.tile_pool(name="wqkv", bufs=3) as w_pool,
            tc.tile_pool(name="wop", bufs=NWOC) as wo_pool,
            tc.tile_pool(name="small", bufs=1) as small_pool,
            tc.tile_pool(name="ps_q", bufs=1, space="PSUM") as ps_q,
            tc.tile_pool(name="ps_k", bufs=1, space="PSUM") as ps_k,
            tc.tile_pool(name="ps_v", bufs=1, space="PSUM") as ps_v,
            tc.tile_pool(name="ps_s", bufs=1, space="PSUM") as ps_s,
            tc.tile_pool(name="ps_o", bufs=2, space="PSUM") as ps_o,
            tc.tile_pool(name="ps_t", bufs=1, space="PSUM") as ps_t,
        ):
            ones_sb = const_pool.tile([128, 1], fp32, tag="ones")
            nc.gpsimd.memset(ones_sb[:], 1.0)
            c64_sb = const_pool.tile([1, 128], fp32, tag="c64")
            nc.gpsimd.memset(c64_sb[:], DS)
            bias_sb = const_pool.tile([1, 1], fp32, tag="bias")
            nc.gpsimd.memset(bias_sb[:], BIAS)
            # warm the Sigmoid activation table off the critical path (the
            # cost model charges a 1.28us table load at first use)
            scr_sb = const_pool.tile([1, 1], fp32, tag="scr")
            nc.scalar.activation(
                scr_sb[:], bias_sb[:], mybir.ActivationFunctionType.Sigmoid,
                bias=bias_sb[:], scale=1.0,
            )

            # ---- input DMAs (SP queue, transfers serialize on the DMA
            # engines in this order; wv last of q/k/v: its post-arrival
            # chain (vm1 -> delta) is the shortest, wo chunks last) ----
            wqx_sb = w_pool.tile([128, XW + ET * F], f8, tag="wqx")
            nc.sync.dma_start(wqx_sb[:], wqx)
            x_sb = wqx_sb[:, :XW].rearrange("p (t s) -> p t s", t=ET)
            wq_sb = wqx_sb[:, XW:].rearrange("p (t f) -> p t f", t=ET)
            wk_sb = w_pool.tile([128, ET, F], f8, tag="wk")
            nc.sync.dma_start(wk_sb[:], wk_r)
            wv_sb = w_pool.tile([128, ET, F], f8, tag="wv")
            nc.sync.dma_start(wv_sb[:], wv_r)
            wo_sb = []  # (tile, et0, net, skip_dlo)
            for k in range(7):
                t = wo_pool.tile([128, HPC, 512], f8, tag="wo")
                nc.sync.dma_start(t[:], wo_r[:, :, k * 512 : (k + 1) * 512])
                # k==6 also skips the delta-residual matmuls: its drain
                # gates w1, whose HWDGE slot gates the final write
                wo_sb.append((t, 4 * k, 4, k == 6))
            t7 = wo_pool.tile([128, HPC, 384], f8, tag="wo7")
            nc.sync.dma_start(t7[:], wo7.rearrange("p (t e) -> p t e", t=HPC))
            wo_sb.append((t7, 28, 3, True))
            t8 = wo_pool.tile([128, HPC, 128], f8, tag="wo8")
            nc.sync.dma_start(t8[:], wo8.rearrange("p (t e) -> p t e", t=HPC))
            wo_sb.append((t8, 31, 1, True))

            # ---- q/k/v projections: weight stationary, x moving ----
            # psum [128d, 4h*8b], accumulated over 32 e-tiles x (hi, lo)
            psq = ps_q.tile([128, HPC * B], fp32, tag="psq")
            psk = ps_k.tile([128, HPC * B], fp32, tag="psk")
            psv = ps_v.tile([128, HPC * B], fp32, tag="psv")
            for w_sb, ps in ((wq_sb, psq), (wk_sb, psk), (wv_sb, psv)):
                for ft in range(HPC):
                    dst = ps[:, ft * B : (ft + 1) * B]
                    for et in range(ET):
                        nc.tensor.matmul(
                            dst,
                            w_sb[:, et, ft * 128 : (ft + 1) * 128],
                            x_sb[:, et, :],
                            start=(et == 0), stop=(et == ET - 1),
                        )

            # ---- closed-form attention (scale QSC on q/k/v psums) ----
            km1 = small_pool.tile([128, HPC * B], fp32, tag="km1")
            nc.vector.tensor_scalar(
                km1[:], psk[:], 1.0 / QSC, -1.0,
                mybir.AluOpType.mult, mybir.AluOpType.add,
            )  # k - 1, exact scale
            vm1 = small_pool.tile([128, HPC * B], fp32, tag="vm1")
            nc.vector.tensor_scalar(
                vm1[:], psv[:], 1.0 / QSC, -1.0,
                mybir.AluOpType.mult, mybir.AluOpType.add,
            )  # v - 1
            qkm = small_pool.tile([128, HPC * B], fp32, tag="qkm")
            nc.vector.tensor_tensor(
                qkm[:], psq[:], km1[:], mybir.AluOpType.mult
            )  # q*(k-1), scale QSC
            # tt[1, 32] = sum_d q*(k-1) = s1 - s0 (scale QSC)
            ps_tt = ps_s.tile([1, HPC * B], fp32, tag="pstt")
            nc.tensor.matmul(ps_tt[:], ones_sb[:], qkm[:], start=True, stop=True)
            p_sb = small_pool.tile([1, HPC * B], fp32, tag="p")
            nc.scalar.activation(
                p_sb[:], ps_tt[:], mybir.ActivationFunctionType.Sigmoid,
                bias=bias_sb[:], scale=SCALE / QSC,
            )
            # broadcast p over partitions, folding the delta fp8 scale:
            # pb[128, 32] = p * DS
            ps_pb = ps_s.tile([128, HPC * B], fp32, tag="pspb")
            nc.tensor.matmul(ps_pb[:], c64_sb[:], p_sb[:], start=True, stop=True)
            dsc = small_pool.tile([128, HPC * B], fp32, tag="dsc")
            nc.vector.tensor_tensor(
                dsc[:], vm1[:], ps_pb[:], mybir.AluOpType.mult
            )  # delta * DS
            dhi = small_pool.tile([128, HPC * B], f8, tag="dhi")
            nc.vector.tensor_copy(dhi[:], dsc[:])
            dhf = small_pool.tile([128, HPC * B], fp32, tag="dhf")
            nc.vector.tensor_copy(dhf[:], dhi[:])
            dlo = small_pool.tile([128, HPC * B], f8, tag="dlo")
            nc.vector.tensor_tensor(
                dlo[:], dsc[:], dhf[:], mybir.AluOpType.subtract
            )

            # ---- out-proj: wo stationary, delta hi/lo moving; psum is
            # out^T [128e, 8b] per e-tile, scale OSC.  Rotating psum tiles
            # let chunk c+1 matmul while chunk c drains to SBUF ----
            out_sb = const_pool.tile([128, ET, B], fp32, tag="osb")
            pso_tail = None
            for wtile, et0, net, skip_dlo in wo_sb:
                if et0 == 28:
                    # the two tail chunks share one psum tile and a single
                    # drain (issued after the last chunk's matmuls), so the
                    # final write waits on one copy instead of two
                    pso_tail = ps_t.tile([128, 4, B], fp32, tag="psot")
                    pso = pso_tail
                elif et0 == 31:
                    pso = pso_tail
                else:
                    pso = ps_o.tile([128, 4, B], fp32, tag="pso")
                for el in range(net):
                    dst = pso[:, (et0 - 28) + el if et0 >= 28 else el, :]
                    for ft in range(HPC):
                        lhs = wtile[:, ft, el * 128 : (el + 1) * 128]
                        # tail chunks skip the delta-residual matmuls:
                        # halves their post-stream PE chain for a sub-1e-4
                        # error contribution on 1/8 of columns
                        nc.tensor.matmul(
                            dst, lhs, dhi[:, ft * B : (ft + 1) * B],
                            start=(ft == 0), stop=(skip_dlo and ft == HPC - 1),
                        )
                        if not skip_dlo:
                            nc.tensor.matmul(
                                dst, lhs, dlo[:, ft * B : (ft + 1) * B],
                                start=False, stop=(ft == HPC - 1),
                            )
                if et0 < 28:
                    nc.vector.tensor_scalar_mul(
                        out_sb[:, et0 : et0 + net, :], pso[:, :net, :], 1.0 / OSC
                    )
                if et0 + net == 28:
                    # et 0..27 ready: big write on Act (idle since the
                    # sigmoid), overlapping the remaining chunks' compute
                    nc.scalar.dma_start(out_r[:, :28], out_sb[:, :28, :])
            nc.vector.tensor_scalar_mul(
                out_sb[:, 28:, :], pso_tail[:], 1.0 / OSC
            )
            # final small write on SP (SEQ decode prepaid after the input
            # dispatches; SP's DGE delay is 650ns vs Act's 784ns, and only
            # HWDGE+DGE+transfer remain after the wait resolves)
            nc.sync.dma_start(out_r[:, 28:], out_sb[:, 28:, :])

    nc.compile()
    return nc


def _get_program(mode=MODE):
    key = "nc_" + mode
    if key not in _CACHE:
        _CACHE[key] = _build_program()
    return _CACHE[key]


def _pack_pmajor(a, tiles):
    """[tiles*128, w] -> [128, tiles*w] partition-major contiguous."""
    w = a.shape[1]
    return np.ascontiguousarray(
        a.reshape(tiles, 128, w).transpose(1, 0, 2).reshape(128, tiles * w)
    )


def _shard_inputs(x, Wq, Wk, Wv, Wo, mode=MODE):
    import ml_dtypes

    f8 = ml_dtypes.float8_e4m3

    def q8(a):
        return np.clip(a, -240.0, 240.0).astype(f8)

    xt = x.reshape(B, E).T * XS  # [E, 8]
    x2 = _pack_pmajor(q8(xt), ET)  # [128, 256]

    in_maps = []
    for c in range(N_CORES):
        rows = slice(c * F, (c + 1) * F)
        wq_p = _pack_pmajor(q8(Wq[rows, :].T * WS), ET)
        wo_p = _pack_pmajor(q8(Wo[:, rows].T * WS), HPC).reshape(128, HPC, E)
        m = {
            "wqx_t": np.ascontiguousarray(np.concatenate([x2, wq_p], axis=1)),
            "wk_t": _pack_pmajor(q8(Wk[rows, :].T * WS), ET),
            "wv_t": _pack_pmajor(q8(Wv[rows, :].T * WS), ET),
            "wo_t": np.ascontiguousarray(wo_p[:, :, :3584]).reshape(128, -1),
            "wo7_t": np.ascontiguousarray(wo_p[:, :, 3584:3968]).reshape(128, -1),
            "wo8_t": np.ascontiguousarray(wo_p[:, :, 3968:]).reshape(128, -1),
        }
        in_maps.append(m)
    return in_maps


def kernel(x, Wq, Wk, Wv, Wo, _trace=False, **_unused):
    from concourse.bass_utils import run_bass_kernel_spmd

    x = np.asarray(x, dtype=np.float32)
    Wq = np.asarray(Wq, dtype=np.float32)
    Wk = np.asarray(Wk, dtype=np.float32)
    Wv = np.asarray(Wv, dtype=np.float32)
    Wo = np.asarray(Wo, dtype=np.float32)

    nc = _get_program()
    in_maps = _shard_inputs(x, Wq, Wk, Wv, Wo)
    core_ids = list(range(N_CORES))

    def _run(trace):
        return run_bass_kernel_spmd(nc, in_maps, core_ids, trace=trace)

    res = None
    if _trace:
        try:
            res = _run(True)
        except Exception:
            # NTFF profiling hooks unavailable in this environment
            res = None
    if res is None:
        # transient device wedges (NRT_EXEC_UNIT_UNRECOVERABLE) heal after
        # a terminal-side reset; tear down the PJRT client and back off
        # before each retry
        import time as _time

        last = None
        for attempt in range(3):
            try:
                res = _run(False)
                break
            except Exception as e:
                last = e
                try:
                    import jax._src.xla_bridge as _xb

                    _xb._clear_backends()
                except Exception:
                    pass
                _time.sleep(15 * (attempt + 1))
        else:
            raise last
    _CACHE["last_results"] = res

    acc = np.zeros((B, E), np.float32)
    for r in res.results:
        # out_p[p, et, b] -> partial[b, et*128 + p]
        buf = np.asarray(r["out_p"], np.float32).reshape(128, ET, B)
        acc += buf.transpose(2, 1, 0).reshape(B, E)
    acc += Wo.sum(axis=1, dtype=np.float32)[None, :]
    return acc.reshape(B, 1, E)
